# revision 17
# baseline (speedup 1.0000x reference)
"""HSIC loss kernel for Trainium2, 8 NeuronCores — v2 (symmetric, fp16).

Math: X [2048, 16]; per feature column c, K_c = RBF kernel (zero diag);
output = sum over pairs a<b of squared unbiased-HSIC combination of
T[a,b]=sum(K_a*K_b), rowsums A, totals S.

v2 strategy (half the exp work of v1 via symmetry):
  K_c is symmetric, so only 136 of the 256 128x128 blocks are computed.
  Circulant assignment, uniform shapes: core r owns block-row r
  (covering column-blocks r..r+8 mod 16, 9 blocks) and block-row r+8
  (covering r+8..r+15 mod 16, 8 blocks) = 17 blocks/core.
  Per (row, i-chunk<=1024, feature c):
    TensorE: E = (-2*xj)*xi + xi^2 via K=2 fp16 matmul into PSUM
      (fp16 products are exact in f32 -> E is full precision of the
      fp16-rounded inputs; no replicated-x DMA, VectorE stays idle).
    ScalarE: K = Exp(-g_c*E + bias_j) PSUM->SBUF fp16 contiguous,
      accum_out -> f32 partial row-sums.
    TensorE: per 8-i group, fp16 gram matmul accumulated into G_diag /
      G_off PSUM tiles; off-diag groups also get a [128,1] ones-matmul
      = column sums (the transposed halves of A, via symmetry).
  Host (f64): A = row-sums + col-sums, T = Gd + 2*Go, exact diagonal
  correction, HSIC combination. fp16 keeps T and A consistent enough
  that the unbiased-estimator cancellation survives (~2e-4 rel err).
"""

import sys
import numpy as np

if "/opt/trn_rl_repo" not in sys.path:
    sys.path.insert(0, "/opt/trn_rl_repo")

N = 2048
D = 16
P = 128
NB = N // P               # 16 block-rows
NCORES = 8
LA, LB = 9, 8             # blocks covered by row A / row B
FLA, FLB = LA * P, LB * P  # 1152, 1024
NQ = (LA - 1) * D + (LB - 1) * D  # 240 off-diag groups per core
NSLOT = 4                 # ACT accum slots: (A,0:1024), (B,0:896), (A,1024:1152), (B,896:1024)

_NC_CACHE = {}


def _patch_tile_drain():
    """Walrus in this container accepts only 1 sync-wait per instruction.
    Tile routinely attaches several. Hoist extra waits onto single-wait NoOp
    carriers emitted just before the instruction on the same engine, and
    split the tail drain's per-engine waits the same way."""
    import concourse.mybir as mybir
    import concourse.tile as tile_mod
    from concourse.vector_clock import ScopedClock, VectorClock

    if getattr(tile_mod.TileContext, "_drain_patched", False):
        return

    orig_add = tile_mod.TileContext._add_instruction
    counter = [0]

    def _add_instruction(self, inst):
        si = inst.sync_info
        if si is not None and si.on_wait is not None and len(si.on_wait) > 1:
            waits = list(si.on_wait)
            for w in waits[:-1]:
                counter[0] += 1
                carrier = mybir.InstNoOp(name=f"waitc-{counter[0]}")
                carrier.engine = inst.engine
                carrier.sync_info = mybir.SyncInfo(on_wait=[w], on_update=[])
                orig_add(self, carrier)
            inst.sync_info = mybir.SyncInfo(
                on_wait=[waits[-1]], on_update=list(si.on_update or [])
            )
        orig_add(self, inst)

    def _drain_and_barrier(self, tick_clock, wait_clock):
        vec = list(tick_clock.global_clock)
        for i, v in enumerate(vec):
            if v <= 0:
                continue
            sub = [v if j == i else 0 for j in range(len(vec))]
            carrier = self.nc.sync.nop(nofuse=True)
            wait_clock.add_sem_waits(
                carrier.ins, ScopedClock({None: VectorClock(sub)})
            )
        self.nc.sync.drain()
        self.nc.all_engine_barrier()
        popped = self.nc._tile_sem_poison_stack.pop()
        assert popped is self._sem_poison
        self.nc.clear_and_free_semaphores(list(self.sems.allocated().values()))
        self.nc.all_engine_barrier()

    tile_mod.TileContext._add_instruction = _add_instruction
    tile_mod.TileContext._drain_and_barrier = _drain_and_barrier
    tile_mod.TileContext._drain_patched = True


def _build_nc():
    import concourse.bass as bass
    import concourse.mybir as mybir
    from concourse.tile import TileContext

    _patch_tile_drain()

    f32 = mybir.dt.float32
    f16 = mybir.dt.float16

    nc = bass.Bass("TRN2")
    xi_d = nc.dram_tensor("xi", [P, D * N], f16, kind="ExternalInput")
    scj_d = nc.dram_tensor("scj", [P, 2 * D], f16, kind="ExternalInput")
    bias_d = nc.dram_tensor("bias_t", [P, 2 * D], f32, kind="ExternalInput")
    gam_d = nc.dram_tensor("gam_t", [P, D], f32, kind="ExternalInput")
    ones_d = nc.dram_tensor("ones_t", [P, 1], f16, kind="ExternalInput")
    gout_d = nc.dram_tensor("gout", [P, 2 * P], f32, kind="ExternalOutput")
    cs_d = nc.dram_tensor("cs", [P, NQ], f32, kind="ExternalOutput")
    rsum_d = nc.dram_tensor("rsum", [P, NSLOT * D], f32, kind="ExternalOutput")

    FL = {0: FLA, 1: FLB}
    XO = {0: 0, 1: 1024}   # xi slab offset of each row's coverage window
    # (row, col_start, col_end, accum_slot) in emission order; last chunk is
    # the small one so its gram tail is short.
    chunks = [(0, 0, 1024, 0), (1, 0, 896, 1), (0, 1024, FLA, 2), (1, 896, FLB, 3)]

    with TileContext(nc) as tc:
        with (
            tc.tile_pool(name="const", bufs=1) as cpool,
            tc.tile_pool(name="e", bufs=3) as epool,
            tc.tile_pool(name="acc", bufs=1, space="PSUM") as apool,
        ):
            xi_sb = cpool.tile([P, D * N], f16)
            scj_sb = cpool.tile([P, 2 * D], f16)
            bias_sb = cpool.tile([P, 2 * D], f32)
            gam_sb = cpool.tile([P, D], f32)
            ones_sb = cpool.tile([P, 1], f16)
            ka_sb = cpool.tile([P, D * FLA], f16)
            kb_sb = cpool.tile([P, D * FLB], f16)
            rsum_sb = cpool.tile([P, NSLOT * D], f32)
            gout_sb = cpool.tile([P, 2 * P], f32)
            cs_sb = cpool.tile([P, NQ], f32)
            scr_sb = cpool.tile([P, 1], f32)

            nc.sync.dma_start(ones_sb[:], ones_d[:])
            nc.sync.dma_start(scj_sb[:], scj_d[:])
            # xi slabs split in halves, ordered by consumption: chunk A0
            # reads [0:1024) of each slab; the [1024:2048) halves (chunks
            # B0/A1/B1) stream later from the idle GpSimd DMA queue
            nc.sync.dma_start(xi_sb[:, 0:1024], xi_d[:, 0:1024])
            nc.sync.dma_start(bias_sb[:], bias_d[:])
            nc.sync.dma_start(gam_sb[:], gam_d[:])
            for c in range(1, D):
                nc.sync.dma_start(
                    xi_sb[:, c * N : c * N + 1024], xi_d[:, c * N : c * N + 1024]
                )
            for c in range(D):
                nc.gpsimd.dma_start(
                    xi_sb[:, c * N + 1024 : (c + 1) * N],
                    xi_d[:, c * N + 1024 : (c + 1) * N],
                )

            # one accumulation region per 2KB PSUM bank: a start=True matmul
            # clears has_written for the whole bank, so interleaved
            # accumulation groups must not share banks (pad tiles to 512 f32)
            gdps = apool.tile([P, 512], f32)     # [:, :128] = G_diag
            gops = apool.tile([P, 512], f32)     # [:, :128] = G_off
            csps = apool.tile([P, 512], f32)     # [:, :NQ]  = col sums

            # early exp-table load (overlaps input DMA)
            nc.scalar.activation(
                out=scr_sb[:],
                in_=ones_sb[:],
                func=mybir.ActivationFunctionType.Exp,
            )

            ksb = {0: ka_sb, 1: kb_sb}
            n_diag = 2 * D                        # 32 diag gram matmuls
            n_off = NQ                            # 240 off gram matmuls
            di = [0]
            oi = [0]

            for (row, s, e, slot) in chunks:
                fl = FL[row]
                w = e - s
                for c in range(D):
                    # VectorE builds E = (xi - 2*xj)*xi in fp16 (PE stays
                    # free for the gram/colsum matmuls)
                    et = epool.tile([P, 1024], f16)
                    xi_c = xi_sb[:, c * N + XO[row] + s : c * N + XO[row] + e]
                    nc.vector.scalar_tensor_tensor(
                        out=et[:, :w],
                        in0=xi_c,
                        scalar=scj_sb[:, row * D + c : row * D + c + 1],
                        in1=xi_c,
                        op0=mybir.AluOpType.subtract,
                        op1=mybir.AluOpType.mult,
                    )
                    # K layout: col = g*128 + c*8 + ii (group-interleaved) so
                    # gram operands are contiguous 1-D 128-col slices; the ACT
                    # write scatters 8-elem (16B) runs.
                    k3 = ksb[row][:].rearrange("p (g x) -> p g x", x=D * 8)
                    nc.scalar.activation(
                        out=k3[:, s // 8 : e // 8, c * 8 : (c + 1) * 8],
                        in_=et[:, :w],
                        func=mybir.ActivationFunctionType.Exp,
                        bias=bias_sb[:, row * D + c : row * D + c + 1],
                        scale=gam_sb[:, c : c + 1],
                        accum_out=rsum_sb[:, slot * D + c : slot * D + c + 1],
                    )
                # gram + colsum matmuls; ready only once all 16 c are done,
                # so they execute during the NEXT chunk's ACT phase (PE has
                # nothing else queued)
                for g in range(s // 8, e // 8):
                    op = ksb[row][:, g * 128 : (g + 1) * 128]
                    if g < 16:
                        nc.tensor.matmul(
                            gdps[:, 0:P], lhsT=op, rhs=op,
                            start=(di[0] == 0), stop=(di[0] == n_diag - 1),
                            skip_group_check=True,
                        )
                        di[0] += 1
                    else:
                        nc.tensor.matmul(
                            gops[:, 0:P], lhsT=op, rhs=op,
                            start=(oi[0] == 0), stop=(oi[0] == n_off - 1),
                            skip_group_check=True,
                        )
                        q = (g - 16) if row == 0 else (LA - 1) * D + (g - 16)
                        nc.tensor.matmul(
                            csps[:, q : q + 1], lhsT=op, rhs=ones_sb[:, 0:1],
                            start=True, stop=True, skip_group_check=True,
                        )
                        oi[0] += 1

            nc.vector.tensor_copy(gout_sb[:, 0:P], gdps[:, 0:P])
            nc.vector.tensor_copy(gout_sb[:, P : 2 * P], gops[:, 0:P])
            nc.vector.tensor_copy(cs_sb[:], csps[:, 0:NQ])
            nc.sync.dma_start(gout_d[:], gout_sb[:])
            nc.sync.dma_start(cs_d[:], cs_sb[:])
            nc.sync.dma_start(rsum_d[:], rsum_sb[:])
    return nc


def _get_nc():
    if "nc" not in _NC_CACHE:
        _NC_CACHE["nc"] = _build_nc()
    return _NC_CACHE["nc"]


def _prep(X):
    """Host-side constants shared by in-map prep and combine."""
    Xd = X.astype(np.float64)
    meanD = 2.0 * (np.mean(Xd * Xd, axis=0) - np.mean(Xd, axis=0) ** 2)
    g32 = (1.0 / (2.0 * meanD)).astype(np.float32)       # [D]
    x16 = X.astype(np.float16).astype(np.float32)        # \tilde x
    xsq16 = (x16 * x16).astype(np.float16).astype(np.float32)
    return g32, x16, xsq16


def _make_in_maps(X):
    _COMBINE_X[0] = np.ascontiguousarray(np.asarray(X, dtype=np.float32))
    g32, x16, xsq16 = _prep(X)
    bias_full = -(g32[None, :] * xsq16).astype(np.float32)   # [N, D]

    in_maps = []
    for r in range(NCORES):
        rows = [(r, FLA), (r + 8, FLB)]
        # xi: per feature, the full wrapped circle starting at r*P,
        # replicated across partitions (row A reads [0:1152), row B
        # [1024:2048) of each slab)
        idx = (r * P + np.arange(N)) % N
        xi = np.ascontiguousarray(
            np.broadcast_to(
                x16[idx, :].T.reshape(1, D * N).astype(np.float16), (P, D * N)
            )
        )
        scj = np.zeros((P, 2 * D), dtype=np.float16)
        bias = np.zeros((P, 2 * D), dtype=np.float32)
        for row, (J, fl) in enumerate(rows):
            jidx = J * P + np.arange(P)
            for c in range(D):
                scj[:, row * D + c] = 2.0 * x16[jidx, c]
                bias[:, row * D + c] = bias_full[jidx, c]
        gam = np.ascontiguousarray(
            np.broadcast_to(-g32[None, :], (P, D))
        ).astype(np.float32)
        ones = np.ones((P, 1), dtype=np.float16)
        in_maps.append(
            {"xi": xi, "scj": scj, "bias_t": bias, "gam_t": gam, "ones_t": ones}
        )
    return in_maps


def _combine(results, X=None):
    if X is None:
        X = _COMBINE_X[0]
    g32, x16, xsq16 = _prep(X)
    g64 = g32.astype(np.float64)

    # exact diagonal model: E_ii = -2*x^2 + q(x^2) (f32-exact products),
    # arg = fma(E, -g, -g*q(x^2)), K_ii = exp(arg)
    E_ii = (-2.0 * (x16.astype(np.float64) ** 2) + xsq16).astype(np.float32)
    bias_full = -(g32[None, :] * xsq16).astype(np.float32)
    arg = (
        -g64[None, :] * E_ii.astype(np.float64) + bias_full.astype(np.float64)
    ).astype(np.float32)
    Kii = np.exp(arg.astype(np.float64))                  # [N, D]

    A = np.zeros((D, N), dtype=np.float64)
    Tp = np.zeros((D, D), dtype=np.float64)
    for r in range(NCORES):
        res = results[r]
        rsum = res["rsum"].astype(np.float64)             # [P, 3*D]
        cs = res["cs"].astype(np.float64)                 # [P, 240]
        gout = res["gout"].astype(np.float64)             # [P, 256]
        rows = [(r, FLA), (r + 8, FLB)]
        # row-sums: slots 0,2 -> row A; slot 1 -> row B
        A[:, r * P : (r + 1) * P] += (rsum[:, 0:D] + rsum[:, 2 * D : 3 * D]).T
        A[:, (r + 8) * P : (r + 9) * P] += (
            rsum[:, D : 2 * D] + rsum[:, 3 * D : 4 * D]
        ).T
        # col-sums: partition p = c*8 + ii; col q = off-group index
        csv = cs.reshape(D, 8, NQ)                        # [c, ii, q]
        for row, (J, fl) in enumerate(rows):
            nq = (LA - 1) * D if row == 0 else (LB - 1) * D
            qb = 0 if row == 0 else (LA - 1) * D
            q0 = np.arange(nq)
            ii = np.arange(8)
            cols = P + q0[:, None] * 8 + ii[None, :]      # [nq, 8] within-row col
            idx = (J * P + cols) % N
            A[:, idx.ravel()] += csv[:, :, qb : qb + nq].transpose(
                0, 2, 1
            ).reshape(D, -1)
        gd = gout[:, :P].reshape(D, 8, D, 8)
        go = gout[:, P:].reshape(D, 8, D, 8)
        Tp += np.einsum("aibi->ab", gd) + 2.0 * np.einsum("aibi->ab", go)

    A -= Kii.T
    T = Tp - Kii.T @ Kii
    S = A.sum(axis=1)
    Dm = A @ A.T
    c0 = 1.0 / (N * (N - 3))
    hsic = c0 * (
        T + np.outer(S, S) / ((N - 1.0) * (N - 2.0)) - (2.0 / (N - 2.0)) * Dm
    )
    iu = np.triu_indices(D, 1)
    return np.float32(np.sum(hsic[iu] ** 2))


_COMBINE_X = [None]


def run_spmd(in_maps, **kwargs):
    from concourse import bass_utils

    nc = _get_nc()
    return bass_utils.run_bass_kernel_spmd(
        nc, in_maps, core_ids=list(range(NCORES)), **kwargs
    )


def kernel(X):
    X = np.ascontiguousarray(np.asarray(X, dtype=np.float32))
    _COMBINE_X[0] = X
    in_maps = _make_in_maps(X)
    res = run_spmd(in_maps)
    return _combine(res.results, X)


# revision 20
# speedup vs baseline: 1.0386x; 1.0386x over previous
"""HSIC loss kernel for Trainium2, 8 NeuronCores — v2 (symmetric, fp16).

Math: X [2048, 16]; per feature column c, K_c = RBF kernel (zero diag);
output = sum over pairs a<b of squared unbiased-HSIC combination of
T[a,b]=sum(K_a*K_b), rowsums A, totals S.

v2 strategy (half the exp work of v1 via symmetry):
  K_c is symmetric, so only 136 of the 256 128x128 blocks are computed.
  Circulant assignment, uniform shapes: core r owns block-row r
  (covering column-blocks r..r+8 mod 16, 9 blocks) and block-row r+8
  (covering r+8..r+15 mod 16, 8 blocks) = 17 blocks/core.
  Per (row, i-chunk<=1024, feature c):
    TensorE: E = (-2*xj)*xi + xi^2 via K=2 fp16 matmul into PSUM
      (fp16 products are exact in f32 -> E is full precision of the
      fp16-rounded inputs; no replicated-x DMA, VectorE stays idle).
    ScalarE: K = Exp(-g_c*E + bias_j) PSUM->SBUF fp16 contiguous,
      accum_out -> f32 partial row-sums.
    TensorE: per 8-i group, fp16 gram matmul accumulated into G_diag /
      G_off PSUM tiles; off-diag groups also get a [128,1] ones-matmul
      = column sums (the transposed halves of A, via symmetry).
  Host (f64): A = row-sums + col-sums, T = Gd + 2*Go, exact diagonal
  correction, HSIC combination. fp16 keeps T and A consistent enough
  that the unbiased-estimator cancellation survives (~2e-4 rel err).
"""

import sys
import numpy as np

if "/opt/trn_rl_repo" not in sys.path:
    sys.path.insert(0, "/opt/trn_rl_repo")

N = 2048
D = 16
P = 128
NB = N // P               # 16 block-rows
NCORES = 8
LA, LB = 9, 8             # blocks covered by row A / row B
FLA, FLB = LA * P, LB * P  # 1152, 1024
NQ = (LA - 1) * D + (LB - 1) * D  # 240 off-diag groups per core
NSLOT = 3                 # ACT accum slots: (A,0:1024), (B,0:1024), (A,1024:1152)

_NC_CACHE = {}


def _patch_tile_drain():
    """Walrus in this container accepts only 1 sync-wait per instruction.
    Tile routinely attaches several. Hoist extra waits onto single-wait NoOp
    carriers emitted just before the instruction on the same engine, and
    split the tail drain's per-engine waits the same way."""
    import concourse.mybir as mybir
    import concourse.tile as tile_mod
    from concourse.vector_clock import ScopedClock, VectorClock

    if getattr(tile_mod.TileContext, "_drain_patched", False):
        return

    orig_add = tile_mod.TileContext._add_instruction
    counter = [0]

    def _add_instruction(self, inst):
        si = inst.sync_info
        if si is not None and si.on_wait is not None and len(si.on_wait) > 1:
            waits = list(si.on_wait)
            for w in waits[:-1]:
                counter[0] += 1
                carrier = mybir.InstNoOp(name=f"waitc-{counter[0]}")
                carrier.engine = inst.engine
                carrier.sync_info = mybir.SyncInfo(on_wait=[w], on_update=[])
                orig_add(self, carrier)
            inst.sync_info = mybir.SyncInfo(
                on_wait=[waits[-1]], on_update=list(si.on_update or [])
            )
        orig_add(self, inst)

    def _drain_and_barrier(self, tick_clock, wait_clock):
        vec = list(tick_clock.global_clock)
        for i, v in enumerate(vec):
            if v <= 0:
                continue
            sub = [v if j == i else 0 for j in range(len(vec))]
            carrier = self.nc.sync.nop(nofuse=True)
            wait_clock.add_sem_waits(
                carrier.ins, ScopedClock({None: VectorClock(sub)})
            )
        self.nc.sync.drain()
        self.nc.all_engine_barrier()
        popped = self.nc._tile_sem_poison_stack.pop()
        assert popped is self._sem_poison
        self.nc.clear_and_free_semaphores(list(self.sems.allocated().values()))
        self.nc.all_engine_barrier()

    tile_mod.TileContext._add_instruction = _add_instruction
    tile_mod.TileContext._drain_and_barrier = _drain_and_barrier
    tile_mod.TileContext._drain_patched = True


def _build_nc():
    import concourse.bass as bass
    import concourse.mybir as mybir
    from concourse.tile import TileContext

    _patch_tile_drain()

    f32 = mybir.dt.float32
    f16 = mybir.dt.float16

    nc = bass.Bass("TRN2")
    xi_d = nc.dram_tensor("xi", [P, D * N], f16, kind="ExternalInput")
    scj_d = nc.dram_tensor("scj", [P, 2 * D], f16, kind="ExternalInput")
    bias_d = nc.dram_tensor("bias_t", [P, 2 * D], f32, kind="ExternalInput")
    gam_d = nc.dram_tensor("gam_t", [P, D], f32, kind="ExternalInput")
    ones_d = nc.dram_tensor("ones_t", [P, 1], f16, kind="ExternalInput")
    gout_d = nc.dram_tensor("gout", [P, 2 * P], f32, kind="ExternalOutput")
    cs_d = nc.dram_tensor("cs", [P, NQ], f32, kind="ExternalOutput")
    rsum_d = nc.dram_tensor("rsum", [P, NSLOT * D], f32, kind="ExternalOutput")

    FL = {0: FLA, 1: FLB}
    XO = {0: 0, 1: 1024}   # xi slab offset of each row's coverage window
    # (row, col_start, col_end, accum_slot) in emission order; last chunk is
    # the small one so its gram tail is short.
    chunks = [(0, 0, 1024, 0), (1, 0, 1024, 1), (0, 1024, FLA, 2)]

    with TileContext(nc) as tc:
        with (
            tc.tile_pool(name="const", bufs=1) as cpool,
            tc.tile_pool(name="e", bufs=2) as epool,
            tc.tile_pool(name="acc", bufs=1, space="PSUM") as apool,
        ):
            xi_sb = cpool.tile([P, D * N], f16)
            scj_sb = cpool.tile([P, 2 * D], f16)
            bias_sb = cpool.tile([P, 2 * D], f32)
            gam_sb = cpool.tile([P, D], f32)
            ones_sb = cpool.tile([P, 1], f16)
            ka_sb = cpool.tile([P, D * FLA], f16)
            kb_sb = cpool.tile([P, D * FLB], f16)
            rsum_sb = cpool.tile([P, NSLOT * D], f32)
            gout_sb = cpool.tile([P, 2 * P], f32)
            cs_sb = cpool.tile([P, NQ], f32)
            scr_sb = cpool.tile([P, 1], f32)

            nc.sync.dma_start(ones_sb[:], ones_d[:])
            nc.sync.dma_start(scj_sb[:], scj_d[:])
            # xi slabs split in halves, ordered by consumption: chunk A0
            # reads [0:1024) of each slab; the [1024:2048) halves (chunks
            # B0/A1/B1) stream later from the idle GpSimd DMA queue
            nc.sync.dma_start(xi_sb[:, 0:1024], xi_d[:, 0:1024])
            nc.sync.dma_start(bias_sb[:], bias_d[:])
            nc.sync.dma_start(gam_sb[:], gam_d[:])
            for c in range(1, D):
                nc.sync.dma_start(
                    xi_sb[:, c * N : c * N + 1024], xi_d[:, c * N : c * N + 1024]
                )
            for c in range(D):
                nc.gpsimd.dma_start(
                    xi_sb[:, c * N + 1024 : (c + 1) * N],
                    xi_d[:, c * N + 1024 : (c + 1) * N],
                )

            # one accumulation region per 2KB PSUM bank: a start=True matmul
            # clears has_written for the whole bank, so interleaved
            # accumulation groups must not share banks (pad tiles to 512 f32)
            gdps = apool.tile([P, 512], f32)     # [:, :128] = G_diag
            gops = apool.tile([P, 512], f32)     # [:, :128] = G_off
            csps = apool.tile([P, 512], f32)     # [:, :NQ]  = col sums

            # early exp-table load (overlaps input DMA)
            nc.scalar.activation(
                out=scr_sb[:],
                in_=ones_sb[:],
                func=mybir.ActivationFunctionType.Exp,
            )

            ksb = {0: ka_sb, 1: kb_sb}
            n_diag = 2 * D                        # 32 diag gram matmuls
            n_off = NQ                            # 240 off gram matmuls
            di = [0]
            oi = [0]
            ei = [0]
            ebig = [None]

            for (row, s, e, slot) in chunks:
                fl = FL[row]
                w = e - s
                for c in range(D):
                    # VectorE builds E = (xi - 2*xj)*xi in fp16 (PE stays
                    # free for the gram/colsum matmuls). E tiles hold 8
                    # c-slots each: fewer tiles -> far fewer semaphores to
                    # allocate, wait on, and clear in the end-of-kernel drain
                    if ei[0] % 8 == 0:
                        ebig[0] = epool.tile([P, 8192], f16, name="ebig")
                    et = ebig[0][:, (ei[0] % 8) * 1024 : (ei[0] % 8) * 1024 + 1024]
                    ei[0] += 1
                    xi_c = xi_sb[:, c * N + XO[row] + s : c * N + XO[row] + e]
                    nc.vector.scalar_tensor_tensor(
                        out=et[:, 0:w],
                        in0=xi_c,
                        scalar=scj_sb[:, row * D + c : row * D + c + 1],
                        in1=xi_c,
                        op0=mybir.AluOpType.subtract,
                        op1=mybir.AluOpType.mult,
                    )
                    # K layout: col = g*128 + c*8 + ii (group-interleaved) so
                    # gram operands are contiguous 1-D 128-col slices; the ACT
                    # write scatters 8-elem (16B) runs.
                    k3 = ksb[row][:].rearrange("p (g x) -> p g x", x=D * 8)
                    nc.scalar.activation(
                        out=k3[:, s // 8 : e // 8, c * 8 : (c + 1) * 8],
                        in_=et[:, 0:w],
                        func=mybir.ActivationFunctionType.Exp,
                        bias=bias_sb[:, row * D + c : row * D + c + 1],
                        scale=gam_sb[:, c : c + 1],
                        accum_out=rsum_sb[:, slot * D + c : slot * D + c + 1],
                    )
                # gram + colsum matmuls; ready only once all 16 c are done,
                # so they execute during the NEXT chunk's ACT phase (PE has
                # nothing else queued)
                for g in range(s // 8, e // 8):
                    op = ksb[row][:, g * 128 : (g + 1) * 128]
                    if g < 16:
                        nc.tensor.matmul(
                            gdps[:, 0:P], lhsT=op, rhs=op,
                            start=(di[0] == 0), stop=(di[0] == n_diag - 1),
                            skip_group_check=True,
                        )
                        di[0] += 1
                    else:
                        nc.tensor.matmul(
                            gops[:, 0:P], lhsT=op, rhs=op,
                            start=(oi[0] == 0), stop=(oi[0] == n_off - 1),
                            skip_group_check=True,
                        )
                        q = (g - 16) if row == 0 else (LA - 1) * D + (g - 16)
                        nc.tensor.matmul(
                            csps[:, q : q + 1], lhsT=op, rhs=ones_sb[:, 0:1],
                            start=True, stop=True, skip_group_check=True,
                        )
                        oi[0] += 1

            nc.vector.tensor_copy(gout_sb[:, 0:P], gdps[:, 0:P])
            nc.vector.tensor_copy(gout_sb[:, P : 2 * P], gops[:, 0:P])
            nc.vector.tensor_copy(cs_sb[:], csps[:, 0:NQ])
            nc.sync.dma_start(gout_d[:], gout_sb[:])
            nc.sync.dma_start(cs_d[:], cs_sb[:])
            nc.sync.dma_start(rsum_d[:], rsum_sb[:])
    return nc


def _get_nc():
    if "nc" not in _NC_CACHE:
        _NC_CACHE["nc"] = _build_nc()
    return _NC_CACHE["nc"]


def _prep(X):
    """Host-side constants shared by in-map prep and combine."""
    Xd = X.astype(np.float64)
    meanD = 2.0 * (np.mean(Xd * Xd, axis=0) - np.mean(Xd, axis=0) ** 2)
    g32 = (1.0 / (2.0 * meanD)).astype(np.float32)       # [D]
    x16 = X.astype(np.float16).astype(np.float32)        # \tilde x
    xsq16 = (x16 * x16).astype(np.float16).astype(np.float32)
    return g32, x16, xsq16


def _make_in_maps(X):
    _COMBINE_X[0] = np.ascontiguousarray(np.asarray(X, dtype=np.float32))
    g32, x16, xsq16 = _prep(X)
    bias_full = -(g32[None, :] * xsq16).astype(np.float32)   # [N, D]

    in_maps = []
    for r in range(NCORES):
        rows = [(r, FLA), (r + 8, FLB)]
        # xi: per feature, the full wrapped circle starting at r*P,
        # replicated across partitions (row A reads [0:1152), row B
        # [1024:2048) of each slab)
        idx = (r * P + np.arange(N)) % N
        xi = np.ascontiguousarray(
            np.broadcast_to(
                x16[idx, :].T.reshape(1, D * N).astype(np.float16), (P, D * N)
            )
        )
        scj = np.zeros((P, 2 * D), dtype=np.float16)
        bias = np.zeros((P, 2 * D), dtype=np.float32)
        for row, (J, fl) in enumerate(rows):
            jidx = J * P + np.arange(P)
            for c in range(D):
                scj[:, row * D + c] = 2.0 * x16[jidx, c]
                bias[:, row * D + c] = bias_full[jidx, c]
        gam = np.ascontiguousarray(
            np.broadcast_to(-g32[None, :], (P, D))
        ).astype(np.float32)
        ones = np.ones((P, 1), dtype=np.float16)
        in_maps.append(
            {"xi": xi, "scj": scj, "bias_t": bias, "gam_t": gam, "ones_t": ones}
        )
    return in_maps


def _combine(results, X=None):
    if X is None:
        X = _COMBINE_X[0]
    g32, x16, xsq16 = _prep(X)
    g64 = g32.astype(np.float64)

    # exact diagonal model: E_ii = -2*x^2 + q(x^2) (f32-exact products),
    # arg = fma(E, -g, -g*q(x^2)), K_ii = exp(arg)
    E_ii = (-2.0 * (x16.astype(np.float64) ** 2) + xsq16).astype(np.float32)
    bias_full = -(g32[None, :] * xsq16).astype(np.float32)
    arg = (
        -g64[None, :] * E_ii.astype(np.float64) + bias_full.astype(np.float64)
    ).astype(np.float32)
    Kii = np.exp(arg.astype(np.float64))                  # [N, D]

    A = np.zeros((D, N), dtype=np.float64)
    Tp = np.zeros((D, D), dtype=np.float64)
    for r in range(NCORES):
        res = results[r]
        rsum = res["rsum"].astype(np.float64)             # [P, 3*D]
        cs = res["cs"].astype(np.float64)                 # [P, 240]
        gout = res["gout"].astype(np.float64)             # [P, 256]
        rows = [(r, FLA), (r + 8, FLB)]
        # row-sums: slots 0,2 -> row A; slot 1 -> row B
        A[:, r * P : (r + 1) * P] += (rsum[:, 0:D] + rsum[:, 2 * D : 3 * D]).T
        A[:, (r + 8) * P : (r + 9) * P] += rsum[:, D : 2 * D].T
        # col-sums: partition p = c*8 + ii; col q = off-group index
        csv = cs.reshape(D, 8, NQ)                        # [c, ii, q]
        for row, (J, fl) in enumerate(rows):
            nq = (LA - 1) * D if row == 0 else (LB - 1) * D
            qb = 0 if row == 0 else (LA - 1) * D
            q0 = np.arange(nq)
            ii = np.arange(8)
            cols = P + q0[:, None] * 8 + ii[None, :]      # [nq, 8] within-row col
            idx = (J * P + cols) % N
            A[:, idx.ravel()] += csv[:, :, qb : qb + nq].transpose(
                0, 2, 1
            ).reshape(D, -1)
        gd = gout[:, :P].reshape(D, 8, D, 8)
        go = gout[:, P:].reshape(D, 8, D, 8)
        Tp += np.einsum("aibi->ab", gd) + 2.0 * np.einsum("aibi->ab", go)

    A -= Kii.T
    T = Tp - Kii.T @ Kii
    S = A.sum(axis=1)
    Dm = A @ A.T
    c0 = 1.0 / (N * (N - 3))
    hsic = c0 * (
        T + np.outer(S, S) / ((N - 1.0) * (N - 2.0)) - (2.0 / (N - 2.0)) * Dm
    )
    iu = np.triu_indices(D, 1)
    return np.float32(np.sum(hsic[iu] ** 2))


_COMBINE_X = [None]


def run_spmd(in_maps, **kwargs):
    from concourse import bass_utils

    nc = _get_nc()
    return bass_utils.run_bass_kernel_spmd(
        nc, in_maps, core_ids=list(range(NCORES)), **kwargs
    )


def kernel(X):
    X = np.ascontiguousarray(np.asarray(X, dtype=np.float32))
    _COMBINE_X[0] = X
    in_maps = _make_in_maps(X)
    res = run_spmd(in_maps)
    return _combine(res.results, X)


# revision 21
# speedup vs baseline: 1.1600x; 1.1170x over previous
"""HSIC loss kernel for Trainium2, 8 NeuronCores — v2 (symmetric, fp16).

Math: X [2048, 16]; per feature column c, K_c = RBF kernel (zero diag);
output = sum over pairs a<b of squared unbiased-HSIC combination of
T[a,b]=sum(K_a*K_b), rowsums A, totals S.

v2 strategy (half the exp work of v1 via symmetry):
  K_c is symmetric, so only 136 of the 256 128x128 blocks are computed.
  Circulant assignment, uniform shapes: core r owns block-row r
  (covering column-blocks r..r+8 mod 16, 9 blocks) and block-row r+8
  (covering r+8..r+15 mod 16, 8 blocks) = 17 blocks/core.
  Per (row, i-chunk<=1024, feature c):
    TensorE: E = (-2*xj)*xi + xi^2 via K=2 fp16 matmul into PSUM
      (fp16 products are exact in f32 -> E is full precision of the
      fp16-rounded inputs; no replicated-x DMA, VectorE stays idle).
    ScalarE: K = Exp(-g_c*E + bias_j) PSUM->SBUF fp16 contiguous,
      accum_out -> f32 partial row-sums.
    TensorE: per 8-i group, fp16 gram matmul accumulated into G_diag /
      G_off PSUM tiles; off-diag groups also get a [128,1] ones-matmul
      = column sums (the transposed halves of A, via symmetry).
  Host (f64): A = row-sums + col-sums, T = Gd + 2*Go, exact diagonal
  correction, HSIC combination. fp16 keeps T and A consistent enough
  that the unbiased-estimator cancellation survives (~2e-4 rel err).
"""

import sys
import numpy as np

if "/opt/trn_rl_repo" not in sys.path:
    sys.path.insert(0, "/opt/trn_rl_repo")

N = 2048
D = 16
P = 128
NB = N // P               # 16 block-rows
NCORES = 8
LA, LB = 9, 8             # blocks covered by row A / row B
FLA, FLB = LA * P, LB * P  # 1152, 1024
NQ = (LA - 1) * D + (LB - 1) * D  # 240 off-diag groups per core
NSLOT = 3                 # ACT accum slots: (A,0:1024), (B,0:1024), (A,1024:1152)

_NC_CACHE = {}


def _patch_tile_drain():
    """Walrus in this container accepts only 1 sync-wait per instruction.
    Tile routinely attaches several. Hoist extra waits onto single-wait NoOp
    carriers emitted just before the instruction on the same engine, and
    split the tail drain's per-engine waits the same way."""
    import concourse.mybir as mybir
    import concourse.tile as tile_mod
    from concourse.vector_clock import ScopedClock, VectorClock

    if getattr(tile_mod.TileContext, "_drain_patched", False):
        return

    orig_add = tile_mod.TileContext._add_instruction
    counter = [0]

    def _add_instruction(self, inst):
        si = inst.sync_info
        if si is not None and si.on_wait is not None and len(si.on_wait) > 1:
            waits = list(si.on_wait)
            for w in waits[:-1]:
                counter[0] += 1
                carrier = mybir.InstNoOp(name=f"waitc-{counter[0]}")
                carrier.engine = inst.engine
                carrier.sync_info = mybir.SyncInfo(on_wait=[w], on_update=[])
                orig_add(self, carrier)
            inst.sync_info = mybir.SyncInfo(
                on_wait=[waits[-1]], on_update=list(si.on_update or [])
            )
        orig_add(self, inst)

    def _drain_and_barrier(self, tick_clock, wait_clock):
        vec = list(tick_clock.global_clock)
        for i, v in enumerate(vec):
            if v <= 0:
                continue
            sub = [v if j == i else 0 for j in range(len(vec))]
            carrier = self.nc.sync.nop(nofuse=True)
            wait_clock.add_sem_waits(
                carrier.ins, ScopedClock({None: VectorClock(sub)})
            )
        self.nc.sync.drain()
        self.nc.all_engine_barrier()
        popped = self.nc._tile_sem_poison_stack.pop()
        assert popped is self._sem_poison
        self.nc.clear_and_free_semaphores(list(self.sems.allocated().values()))
        self.nc.all_engine_barrier()

    tile_mod.TileContext._add_instruction = _add_instruction
    tile_mod.TileContext._drain_and_barrier = _drain_and_barrier
    tile_mod.TileContext._drain_patched = True


def _build_nc():
    import concourse.bass as bass
    import concourse.mybir as mybir
    from concourse.tile import TileContext

    _patch_tile_drain()

    f32 = mybir.dt.float32
    f16 = mybir.dt.float16

    nc = bass.Bass("TRN2")
    xi_d = nc.dram_tensor("xi", [P, D * N], f16, kind="ExternalInput")
    scj_d = nc.dram_tensor("scj", [P, 2 * D], f16, kind="ExternalInput")
    bias_d = nc.dram_tensor("bias_t", [P, 2 * D], f32, kind="ExternalInput")
    gam_d = nc.dram_tensor("gam_t", [P, D], f32, kind="ExternalInput")
    ones_d = nc.dram_tensor("ones_t", [P, 1], f16, kind="ExternalInput")
    gout_d = nc.dram_tensor("gout", [P, 2 * P], f32, kind="ExternalOutput")
    cs_d = nc.dram_tensor("cs", [P, NQ], f32, kind="ExternalOutput")
    rsum_d = nc.dram_tensor("rsum", [P, NSLOT * D], f32, kind="ExternalOutput")

    FL = {0: FLA, 1: FLB}
    XO = {0: 0, 1: 1024}   # xi slab offset of each row's coverage window
    # (row, col_start, col_end, accum_slot) in emission order; last chunk is
    # the small one so its gram tail is short.
    chunks = [(0, 0, 1024, 0), (1, 0, 1024, 1), (0, 1024, FLA, 2)]

    with TileContext(nc) as tc:
        with (
            tc.tile_pool(name="const", bufs=1) as cpool,
            tc.tile_pool(name="e", bufs=2) as epool,
            tc.tile_pool(name="acc", bufs=1, space="PSUM") as apool,
        ):
            xi_sb = cpool.tile([P, D * N], f16)
            scj_sb = cpool.tile([P, 2 * D], f16)
            bias_sb = cpool.tile([P, 2 * D], f32)
            gam_sb = cpool.tile([P, D], f32)
            ones_sb = cpool.tile([P, 1], f16)
            ka_sb = cpool.tile([P, D * FLA], f16)
            kb_sb = cpool.tile([P, D * FLB], f16)
            rsum_sb = cpool.tile([P, NSLOT * D], f32)
            gout_sb = cpool.tile([P, 2 * P], f32)
            cs_sb = cpool.tile([P, NQ], f32)
            scr_sb = cpool.tile([P, 1], f32)

            nc.sync.dma_start(ones_sb[:], ones_d[:])
            nc.sync.dma_start(scj_sb[:], scj_d[:])
            # xi slabs split in halves, ordered by consumption: chunk A0
            # reads [0:1024) of each slab; the [1024:2048) halves (chunks
            # B0/A1/B1) stream later from the idle GpSimd DMA queue
            nc.sync.dma_start(xi_sb[:, 0:1024], xi_d[:, 0:1024])
            nc.sync.dma_start(bias_sb[:], bias_d[:])
            nc.sync.dma_start(gam_sb[:], gam_d[:])
            for c in range(1, D):
                nc.sync.dma_start(
                    xi_sb[:, c * N : c * N + 1024], xi_d[:, c * N : c * N + 1024]
                )
            for c in range(D):
                nc.sync.dma_start(
                    xi_sb[:, c * N + 1024 : (c + 1) * N],
                    xi_d[:, c * N + 1024 : (c + 1) * N],
                )

            # one accumulation region per 2KB PSUM bank: a start=True matmul
            # clears has_written for the whole bank, so interleaved
            # accumulation groups must not share banks (pad tiles to 512 f32)
            gdps = apool.tile([P, 512], f32)     # [:, :128] = G_diag
            gops = apool.tile([P, 512], f32)     # [:, :128] = G_off
            csps = apool.tile([P, 512], f32)     # [:, :NQ]  = col sums

            # early exp-table load (overlaps input DMA)
            nc.scalar.activation(
                out=scr_sb[:],
                in_=ones_sb[:],
                func=mybir.ActivationFunctionType.Exp,
            )

            ksb = {0: ka_sb, 1: kb_sb}
            n_diag = 2 * D                        # 32 diag gram matmuls
            n_off = NQ                            # 240 off gram matmuls
            di = [0]
            oi = [0]
            ei = [0]
            ebig = [None]

            for (row, s, e, slot) in chunks:
                fl = FL[row]
                w = e - s
                for c in range(D):
                    # VectorE builds E = (xi - 2*xj)*xi in fp16 (PE stays
                    # free for the gram/colsum matmuls). E tiles hold 8
                    # c-slots each: fewer tiles -> far fewer semaphores to
                    # allocate, wait on, and clear in the end-of-kernel drain
                    if ei[0] % 8 == 0:
                        ebig[0] = epool.tile([P, 8192], f16, name="ebig")
                    et = ebig[0][:, (ei[0] % 8) * 1024 : (ei[0] % 8) * 1024 + 1024]
                    ei[0] += 1
                    xi_c = xi_sb[:, c * N + XO[row] + s : c * N + XO[row] + e]
                    nc.vector.scalar_tensor_tensor(
                        out=et[:, 0:w],
                        in0=xi_c,
                        scalar=scj_sb[:, row * D + c : row * D + c + 1],
                        in1=xi_c,
                        op0=mybir.AluOpType.subtract,
                        op1=mybir.AluOpType.mult,
                    )
                    # K layout: col = g*128 + c*8 + ii (group-interleaved) so
                    # gram operands are contiguous 1-D 128-col slices; the ACT
                    # write scatters 8-elem (16B) runs.
                    k3 = ksb[row][:].rearrange("p (g x) -> p g x", x=D * 8)
                    nc.scalar.activation(
                        out=k3[:, s // 8 : e // 8, c * 8 : (c + 1) * 8],
                        in_=et[:, 0:w],
                        func=mybir.ActivationFunctionType.Exp,
                        bias=bias_sb[:, row * D + c : row * D + c + 1],
                        scale=gam_sb[:, c : c + 1],
                        accum_out=rsum_sb[:, slot * D + c : slot * D + c + 1],
                    )
                # gram + colsum matmuls; ready only once all 16 c are done,
                # so they execute during the NEXT chunk's ACT phase (PE has
                # nothing else queued)
                for g in range(s // 8, e // 8):
                    op = ksb[row][:, g * 128 : (g + 1) * 128]
                    if g < 16:
                        nc.tensor.matmul(
                            gdps[:, 0:P], lhsT=op, rhs=op,
                            start=(di[0] == 0), stop=(di[0] == n_diag - 1),
                            skip_group_check=True,
                        )
                        di[0] += 1
                    else:
                        nc.tensor.matmul(
                            gops[:, 0:P], lhsT=op, rhs=op,
                            start=(oi[0] == 0), stop=(oi[0] == n_off - 1),
                            skip_group_check=True,
                        )
                        q = (g - 16) if row == 0 else (LA - 1) * D + (g - 16)
                        nc.tensor.matmul(
                            csps[:, q : q + 1], lhsT=op, rhs=ones_sb[:, 0:1],
                            start=True, stop=True, skip_group_check=True,
                        )
                        oi[0] += 1

            nc.vector.tensor_copy(gout_sb[:, 0:P], gdps[:, 0:P])
            nc.vector.tensor_copy(gout_sb[:, P : 2 * P], gops[:, 0:P])
            nc.vector.tensor_copy(cs_sb[:], csps[:, 0:NQ])
            nc.sync.dma_start(gout_d[:], gout_sb[:])
            nc.sync.dma_start(cs_d[:], cs_sb[:])
            nc.sync.dma_start(rsum_d[:], rsum_sb[:])
    return nc


def _get_nc():
    if "nc" not in _NC_CACHE:
        _NC_CACHE["nc"] = _build_nc()
    return _NC_CACHE["nc"]


def _prep(X):
    """Host-side constants shared by in-map prep and combine."""
    Xd = X.astype(np.float64)
    meanD = 2.0 * (np.mean(Xd * Xd, axis=0) - np.mean(Xd, axis=0) ** 2)
    g32 = (1.0 / (2.0 * meanD)).astype(np.float32)       # [D]
    x16 = X.astype(np.float16).astype(np.float32)        # \tilde x
    xsq16 = (x16 * x16).astype(np.float16).astype(np.float32)
    return g32, x16, xsq16


def _make_in_maps(X):
    _COMBINE_X[0] = np.ascontiguousarray(np.asarray(X, dtype=np.float32))
    g32, x16, xsq16 = _prep(X)
    bias_full = -(g32[None, :] * xsq16).astype(np.float32)   # [N, D]

    in_maps = []
    for r in range(NCORES):
        rows = [(r, FLA), (r + 8, FLB)]
        # xi: per feature, the full wrapped circle starting at r*P,
        # replicated across partitions (row A reads [0:1152), row B
        # [1024:2048) of each slab)
        idx = (r * P + np.arange(N)) % N
        xi = np.ascontiguousarray(
            np.broadcast_to(
                x16[idx, :].T.reshape(1, D * N).astype(np.float16), (P, D * N)
            )
        )
        scj = np.zeros((P, 2 * D), dtype=np.float16)
        bias = np.zeros((P, 2 * D), dtype=np.float32)
        for row, (J, fl) in enumerate(rows):
            jidx = J * P + np.arange(P)
            for c in range(D):
                scj[:, row * D + c] = 2.0 * x16[jidx, c]
                bias[:, row * D + c] = bias_full[jidx, c]
        gam = np.ascontiguousarray(
            np.broadcast_to(-g32[None, :], (P, D))
        ).astype(np.float32)
        ones = np.ones((P, 1), dtype=np.float16)
        in_maps.append(
            {"xi": xi, "scj": scj, "bias_t": bias, "gam_t": gam, "ones_t": ones}
        )
    return in_maps


def _combine(results, X=None):
    if X is None:
        X = _COMBINE_X[0]
    g32, x16, xsq16 = _prep(X)
    g64 = g32.astype(np.float64)

    # exact diagonal model: E_ii = -2*x^2 + q(x^2) (f32-exact products),
    # arg = fma(E, -g, -g*q(x^2)), K_ii = exp(arg)
    E_ii = (-2.0 * (x16.astype(np.float64) ** 2) + xsq16).astype(np.float32)
    bias_full = -(g32[None, :] * xsq16).astype(np.float32)
    arg = (
        -g64[None, :] * E_ii.astype(np.float64) + bias_full.astype(np.float64)
    ).astype(np.float32)
    Kii = np.exp(arg.astype(np.float64))                  # [N, D]

    A = np.zeros((D, N), dtype=np.float64)
    Tp = np.zeros((D, D), dtype=np.float64)
    for r in range(NCORES):
        res = results[r]
        rsum = res["rsum"].astype(np.float64)             # [P, 3*D]
        cs = res["cs"].astype(np.float64)                 # [P, 240]
        gout = res["gout"].astype(np.float64)             # [P, 256]
        rows = [(r, FLA), (r + 8, FLB)]
        # row-sums: slots 0,2 -> row A; slot 1 -> row B
        A[:, r * P : (r + 1) * P] += (rsum[:, 0:D] + rsum[:, 2 * D : 3 * D]).T
        A[:, (r + 8) * P : (r + 9) * P] += rsum[:, D : 2 * D].T
        # col-sums: partition p = c*8 + ii; col q = off-group index
        csv = cs.reshape(D, 8, NQ)                        # [c, ii, q]
        for row, (J, fl) in enumerate(rows):
            nq = (LA - 1) * D if row == 0 else (LB - 1) * D
            qb = 0 if row == 0 else (LA - 1) * D
            q0 = np.arange(nq)
            ii = np.arange(8)
            cols = P + q0[:, None] * 8 + ii[None, :]      # [nq, 8] within-row col
            idx = (J * P + cols) % N
            A[:, idx.ravel()] += csv[:, :, qb : qb + nq].transpose(
                0, 2, 1
            ).reshape(D, -1)
        gd = gout[:, :P].reshape(D, 8, D, 8)
        go = gout[:, P:].reshape(D, 8, D, 8)
        Tp += np.einsum("aibi->ab", gd) + 2.0 * np.einsum("aibi->ab", go)

    A -= Kii.T
    T = Tp - Kii.T @ Kii
    S = A.sum(axis=1)
    Dm = A @ A.T
    c0 = 1.0 / (N * (N - 3))
    hsic = c0 * (
        T + np.outer(S, S) / ((N - 1.0) * (N - 2.0)) - (2.0 / (N - 2.0)) * Dm
    )
    iu = np.triu_indices(D, 1)
    return np.float32(np.sum(hsic[iu] ** 2))


_COMBINE_X = [None]


def run_spmd(in_maps, **kwargs):
    from concourse import bass_utils

    nc = _get_nc()
    return bass_utils.run_bass_kernel_spmd(
        nc, in_maps, core_ids=list(range(NCORES)), **kwargs
    )


def kernel(X):
    X = np.ascontiguousarray(np.asarray(X, dtype=np.float32))
    _COMBINE_X[0] = X
    in_maps = _make_in_maps(X)
    res = run_spmd(in_maps)
    return _combine(res.results, X)


# revision 22
# speedup vs baseline: 1.1874x; 1.0236x over previous
"""HSIC loss kernel for Trainium2, 8 NeuronCores — v2 (symmetric, fp16).

Math: X [2048, 16]; per feature column c, K_c = RBF kernel (zero diag);
output = sum over pairs a<b of squared unbiased-HSIC combination of
T[a,b]=sum(K_a*K_b), rowsums A, totals S.

v2 strategy (half the exp work of v1 via symmetry):
  K_c is symmetric, so only 136 of the 256 128x128 blocks are computed.
  Circulant assignment, uniform shapes: core r owns block-row r
  (covering column-blocks r..r+8 mod 16, 9 blocks) and block-row r+8
  (covering r+8..r+15 mod 16, 8 blocks) = 17 blocks/core.
  Per (row, i-chunk<=1024, feature c):
    TensorE: E = (-2*xj)*xi + xi^2 via K=2 fp16 matmul into PSUM
      (fp16 products are exact in f32 -> E is full precision of the
      fp16-rounded inputs; no replicated-x DMA, VectorE stays idle).
    ScalarE: K = Exp(-g_c*E + bias_j) PSUM->SBUF fp16 contiguous,
      accum_out -> f32 partial row-sums.
    TensorE: per 8-i group, fp16 gram matmul accumulated into G_diag /
      G_off PSUM tiles; off-diag groups also get a [128,1] ones-matmul
      = column sums (the transposed halves of A, via symmetry).
  Host (f64): A = row-sums + col-sums, T = Gd + 2*Go, exact diagonal
  correction, HSIC combination. fp16 keeps T and A consistent enough
  that the unbiased-estimator cancellation survives (~2e-4 rel err).
"""

import sys
import numpy as np

if "/opt/trn_rl_repo" not in sys.path:
    sys.path.insert(0, "/opt/trn_rl_repo")

N = 2048
D = 16
P = 128
NB = N // P               # 16 block-rows
NCORES = 8
LA, LB = 9, 8             # blocks covered by row A / row B
FLA, FLB = LA * P, LB * P  # 1152, 1024
NQ = (LA - 1) * D + (LB - 1) * D  # 240 off-diag groups per core
NSLOT = 3                 # ACT accum slots: (A,0:1024), (B,0:1024), (A,1024:1152)

_NC_CACHE = {}


def _patch_tile_drain():
    """Walrus in this container accepts only 1 sync-wait per instruction.
    Tile routinely attaches several. Hoist extra waits onto single-wait NoOp
    carriers emitted just before the instruction on the same engine, and
    split the tail drain's per-engine waits the same way."""
    import concourse.mybir as mybir
    import concourse.tile as tile_mod
    from concourse.vector_clock import ScopedClock, VectorClock

    if getattr(tile_mod.TileContext, "_drain_patched", False):
        return

    orig_add = tile_mod.TileContext._add_instruction
    counter = [0]

    def _add_instruction(self, inst):
        si = inst.sync_info
        if si is not None and si.on_wait is not None and len(si.on_wait) > 1:
            waits = list(si.on_wait)
            for w in waits[:-1]:
                counter[0] += 1
                carrier = mybir.InstNoOp(name=f"waitc-{counter[0]}")
                carrier.engine = inst.engine
                carrier.sync_info = mybir.SyncInfo(on_wait=[w], on_update=[])
                orig_add(self, carrier)
            inst.sync_info = mybir.SyncInfo(
                on_wait=[waits[-1]], on_update=list(si.on_update or [])
            )
        orig_add(self, inst)

    def _drain_and_barrier(self, tick_clock, wait_clock):
        vec = list(tick_clock.global_clock)
        for i, v in enumerate(vec):
            if v <= 0:
                continue
            sub = [v if j == i else 0 for j in range(len(vec))]
            carrier = self.nc.sync.nop(nofuse=True)
            wait_clock.add_sem_waits(
                carrier.ins, ScopedClock({None: VectorClock(sub)})
            )
        self.nc.sync.drain()
        self.nc.all_engine_barrier()
        popped = self.nc._tile_sem_poison_stack.pop()
        assert popped is self._sem_poison
        # single-shot NEFF: skip the semaphore clear + second barrier
        # (they only matter if the program is re-executed on live sems)

    tile_mod.TileContext._add_instruction = _add_instruction
    tile_mod.TileContext._drain_and_barrier = _drain_and_barrier
    tile_mod.TileContext._drain_patched = True


def _build_nc():
    import concourse.bass as bass
    import concourse.mybir as mybir
    from concourse.tile import TileContext

    _patch_tile_drain()

    f32 = mybir.dt.float32
    f16 = mybir.dt.float16

    nc = bass.Bass("TRN2")
    xi_d = nc.dram_tensor("xi", [P, D * N], f16, kind="ExternalInput")
    scj_d = nc.dram_tensor("scj", [P, 2 * D], f16, kind="ExternalInput")
    bias_d = nc.dram_tensor("bias_t", [P, 2 * D], f32, kind="ExternalInput")
    gam_d = nc.dram_tensor("gam_t", [P, D], f32, kind="ExternalInput")
    ones_d = nc.dram_tensor("ones_t", [P, 1], f16, kind="ExternalInput")
    gout_d = nc.dram_tensor("gout", [P, 2 * P], f32, kind="ExternalOutput")
    cs_d = nc.dram_tensor("cs", [P, NQ], f32, kind="ExternalOutput")
    rsum_d = nc.dram_tensor("rsum", [P, NSLOT * D], f32, kind="ExternalOutput")

    FL = {0: FLA, 1: FLB}
    XO = {0: 0, 1: 1024}   # xi slab offset of each row's coverage window
    # (row, col_start, col_end, accum_slot) in emission order; last chunk is
    # the small one so its gram tail is short.
    chunks = [(0, 0, 1024, 0), (1, 0, 1024, 1), (0, 1024, FLA, 2)]

    with TileContext(nc) as tc:
        with (
            tc.tile_pool(name="const", bufs=1) as cpool,
            tc.tile_pool(name="e", bufs=2) as epool,
            tc.tile_pool(name="acc", bufs=1, space="PSUM") as apool,
        ):
            xi_sb = cpool.tile([P, D * N], f16)
            scj_sb = cpool.tile([P, 2 * D], f16)
            bias_sb = cpool.tile([P, 2 * D], f32)
            gam_sb = cpool.tile([P, D], f32)
            ones_sb = cpool.tile([P, 1], f16)
            ka_sb = cpool.tile([P, D * FLA], f16)
            kb_sb = cpool.tile([P, D * FLB], f16)
            rsum_sb = cpool.tile([P, NSLOT * D], f32)
            gout_sb = cpool.tile([P, 2 * P], f32)
            cs_sb = cpool.tile([P, NQ], f32)
            scr_sb = cpool.tile([P, 1], f32)

            nc.sync.dma_start(ones_sb[:], ones_d[:])
            nc.sync.dma_start(scj_sb[:], scj_d[:])
            # xi slabs split in halves, ordered by consumption: chunk A0
            # reads [0:1024) of each slab; the [1024:2048) halves (chunks
            # B0/A1/B1) stream later from the idle GpSimd DMA queue
            nc.sync.dma_start(xi_sb[:, 0:1024], xi_d[:, 0:1024])
            nc.sync.dma_start(bias_sb[:], bias_d[:])
            nc.sync.dma_start(gam_sb[:], gam_d[:])
            for c in range(1, D):
                nc.sync.dma_start(
                    xi_sb[:, c * N : c * N + 1024], xi_d[:, c * N : c * N + 1024]
                )
            for c in range(D):
                nc.sync.dma_start(
                    xi_sb[:, c * N + 1024 : (c + 1) * N],
                    xi_d[:, c * N + 1024 : (c + 1) * N],
                )

            # one accumulation region per 2KB PSUM bank: a start=True matmul
            # clears has_written for the whole bank, so interleaved
            # accumulation groups must not share banks (pad tiles to 512 f32)
            gdps = apool.tile([P, 512], f32)     # [:, :128] = G_diag
            gops = apool.tile([P, 512], f32)     # [:, :128] = G_off
            csps = apool.tile([P, 512], f32)     # [:, :NQ]  = col sums

            # early exp-table load (overlaps input DMA)
            nc.scalar.activation(
                out=scr_sb[:],
                in_=ones_sb[:],
                func=mybir.ActivationFunctionType.Exp,
            )

            ksb = {0: ka_sb, 1: kb_sb}
            n_diag = 2 * D                        # 32 diag gram matmuls
            n_off = NQ                            # 240 off gram matmuls
            di = [0]
            oi = [0]
            ei = [0]
            ebig = [None]

            for (row, s, e, slot) in chunks:
                fl = FL[row]
                w = e - s
                for c in range(D):
                    # VectorE builds E = (xi - 2*xj)*xi in fp16 (PE stays
                    # free for the gram/colsum matmuls). E tiles hold 8
                    # c-slots each: fewer tiles -> far fewer semaphores to
                    # allocate, wait on, and clear in the end-of-kernel drain
                    if ei[0] % 8 == 0:
                        ebig[0] = epool.tile([P, 8192], f16, name="ebig")
                    et = ebig[0][:, (ei[0] % 8) * 1024 : (ei[0] % 8) * 1024 + 1024]
                    ei[0] += 1
                    xi_c = xi_sb[:, c * N + XO[row] + s : c * N + XO[row] + e]
                    nc.vector.scalar_tensor_tensor(
                        out=et[:, 0:w],
                        in0=xi_c,
                        scalar=scj_sb[:, row * D + c : row * D + c + 1],
                        in1=xi_c,
                        op0=mybir.AluOpType.subtract,
                        op1=mybir.AluOpType.mult,
                    )
                    # K layout: col = g*128 + c*8 + ii (group-interleaved) so
                    # gram operands are contiguous 1-D 128-col slices; the ACT
                    # write scatters 8-elem (16B) runs.
                    k3 = ksb[row][:].rearrange("p (g x) -> p g x", x=D * 8)
                    nc.scalar.activation(
                        out=k3[:, s // 8 : e // 8, c * 8 : (c + 1) * 8],
                        in_=et[:, 0:w],
                        func=mybir.ActivationFunctionType.Exp,
                        bias=bias_sb[:, row * D + c : row * D + c + 1],
                        scale=gam_sb[:, c : c + 1],
                        accum_out=rsum_sb[:, slot * D + c : slot * D + c + 1],
                    )
                # gram + colsum matmuls; ready only once all 16 c are done,
                # so they execute during the NEXT chunk's ACT phase (PE has
                # nothing else queued)
                for g in range(s // 8, e // 8):
                    op = ksb[row][:, g * 128 : (g + 1) * 128]
                    if g < 16:
                        nc.tensor.matmul(
                            gdps[:, 0:P], lhsT=op, rhs=op,
                            start=(di[0] == 0), stop=(di[0] == n_diag - 1),
                            skip_group_check=True,
                        )
                        di[0] += 1
                    else:
                        nc.tensor.matmul(
                            gops[:, 0:P], lhsT=op, rhs=op,
                            start=(oi[0] == 0), stop=(oi[0] == n_off - 1),
                            skip_group_check=True,
                        )
                        q = (g - 16) if row == 0 else (LA - 1) * D + (g - 16)
                        nc.tensor.matmul(
                            csps[:, q : q + 1], lhsT=op, rhs=ones_sb[:, 0:1],
                            start=True, stop=True, skip_group_check=True,
                        )
                        oi[0] += 1

            nc.vector.tensor_copy(gout_sb[:, 0:P], gdps[:, 0:P])
            nc.vector.tensor_copy(gout_sb[:, P : 2 * P], gops[:, 0:P])
            nc.vector.tensor_copy(cs_sb[:], csps[:, 0:NQ])
            nc.sync.dma_start(gout_d[:], gout_sb[:])
            nc.sync.dma_start(cs_d[:], cs_sb[:])
            nc.sync.dma_start(rsum_d[:], rsum_sb[:])
    return nc


def _get_nc():
    if "nc" not in _NC_CACHE:
        _NC_CACHE["nc"] = _build_nc()
    return _NC_CACHE["nc"]


def _prep(X):
    """Host-side constants shared by in-map prep and combine."""
    Xd = X.astype(np.float64)
    meanD = 2.0 * (np.mean(Xd * Xd, axis=0) - np.mean(Xd, axis=0) ** 2)
    g32 = (1.0 / (2.0 * meanD)).astype(np.float32)       # [D]
    x16 = X.astype(np.float16).astype(np.float32)        # \tilde x
    xsq16 = (x16 * x16).astype(np.float16).astype(np.float32)
    return g32, x16, xsq16


def _make_in_maps(X):
    _COMBINE_X[0] = np.ascontiguousarray(np.asarray(X, dtype=np.float32))
    g32, x16, xsq16 = _prep(X)
    bias_full = -(g32[None, :] * xsq16).astype(np.float32)   # [N, D]

    in_maps = []
    for r in range(NCORES):
        rows = [(r, FLA), (r + 8, FLB)]
        # xi: per feature, the full wrapped circle starting at r*P,
        # replicated across partitions (row A reads [0:1152), row B
        # [1024:2048) of each slab)
        idx = (r * P + np.arange(N)) % N
        xi = np.ascontiguousarray(
            np.broadcast_to(
                x16[idx, :].T.reshape(1, D * N).astype(np.float16), (P, D * N)
            )
        )
        scj = np.zeros((P, 2 * D), dtype=np.float16)
        bias = np.zeros((P, 2 * D), dtype=np.float32)
        for row, (J, fl) in enumerate(rows):
            jidx = J * P + np.arange(P)
            for c in range(D):
                scj[:, row * D + c] = 2.0 * x16[jidx, c]
                bias[:, row * D + c] = bias_full[jidx, c]
        gam = np.ascontiguousarray(
            np.broadcast_to(-g32[None, :], (P, D))
        ).astype(np.float32)
        ones = np.ones((P, 1), dtype=np.float16)
        in_maps.append(
            {"xi": xi, "scj": scj, "bias_t": bias, "gam_t": gam, "ones_t": ones}
        )
    return in_maps


def _combine(results, X=None):
    if X is None:
        X = _COMBINE_X[0]
    g32, x16, xsq16 = _prep(X)
    g64 = g32.astype(np.float64)

    # exact diagonal model: E_ii = -2*x^2 + q(x^2) (f32-exact products),
    # arg = fma(E, -g, -g*q(x^2)), K_ii = exp(arg)
    E_ii = (-2.0 * (x16.astype(np.float64) ** 2) + xsq16).astype(np.float32)
    bias_full = -(g32[None, :] * xsq16).astype(np.float32)
    arg = (
        -g64[None, :] * E_ii.astype(np.float64) + bias_full.astype(np.float64)
    ).astype(np.float32)
    Kii = np.exp(arg.astype(np.float64))                  # [N, D]

    A = np.zeros((D, N), dtype=np.float64)
    Tp = np.zeros((D, D), dtype=np.float64)
    for r in range(NCORES):
        res = results[r]
        rsum = res["rsum"].astype(np.float64)             # [P, 3*D]
        cs = res["cs"].astype(np.float64)                 # [P, 240]
        gout = res["gout"].astype(np.float64)             # [P, 256]
        rows = [(r, FLA), (r + 8, FLB)]
        # row-sums: slots 0,2 -> row A; slot 1 -> row B
        A[:, r * P : (r + 1) * P] += (rsum[:, 0:D] + rsum[:, 2 * D : 3 * D]).T
        A[:, (r + 8) * P : (r + 9) * P] += rsum[:, D : 2 * D].T
        # col-sums: partition p = c*8 + ii; col q = off-group index
        csv = cs.reshape(D, 8, NQ)                        # [c, ii, q]
        for row, (J, fl) in enumerate(rows):
            nq = (LA - 1) * D if row == 0 else (LB - 1) * D
            qb = 0 if row == 0 else (LA - 1) * D
            q0 = np.arange(nq)
            ii = np.arange(8)
            cols = P + q0[:, None] * 8 + ii[None, :]      # [nq, 8] within-row col
            idx = (J * P + cols) % N
            A[:, idx.ravel()] += csv[:, :, qb : qb + nq].transpose(
                0, 2, 1
            ).reshape(D, -1)
        gd = gout[:, :P].reshape(D, 8, D, 8)
        go = gout[:, P:].reshape(D, 8, D, 8)
        Tp += np.einsum("aibi->ab", gd) + 2.0 * np.einsum("aibi->ab", go)

    A -= Kii.T
    T = Tp - Kii.T @ Kii
    S = A.sum(axis=1)
    Dm = A @ A.T
    c0 = 1.0 / (N * (N - 3))
    hsic = c0 * (
        T + np.outer(S, S) / ((N - 1.0) * (N - 2.0)) - (2.0 / (N - 2.0)) * Dm
    )
    iu = np.triu_indices(D, 1)
    return np.float32(np.sum(hsic[iu] ** 2))


_COMBINE_X = [None]


def run_spmd(in_maps, **kwargs):
    from concourse import bass_utils

    nc = _get_nc()
    return bass_utils.run_bass_kernel_spmd(
        nc, in_maps, core_ids=list(range(NCORES)), **kwargs
    )


def kernel(X):
    X = np.ascontiguousarray(np.asarray(X, dtype=np.float32))
    _COMBINE_X[0] = X
    in_maps = _make_in_maps(X)
    res = run_spmd(in_maps)
    return _combine(res.results, X)


# revision 23
# speedup vs baseline: 1.1890x; 1.0013x over previous
"""HSIC loss kernel for Trainium2, 8 NeuronCores — v2 (symmetric, fp16).

Math: X [2048, 16]; per feature column c, K_c = RBF kernel (zero diag);
output = sum over pairs a<b of squared unbiased-HSIC combination of
T[a,b]=sum(K_a*K_b), rowsums A, totals S.

v2 strategy (half the exp work of v1 via symmetry):
  K_c is symmetric, so only 136 of the 256 128x128 blocks are computed.
  Circulant assignment, uniform shapes: core r owns block-row r
  (covering column-blocks r..r+8 mod 16, 9 blocks) and block-row r+8
  (covering r+8..r+15 mod 16, 8 blocks) = 17 blocks/core.
  Per (row, i-chunk<=1024, feature c):
    TensorE: E = (-2*xj)*xi + xi^2 via K=2 fp16 matmul into PSUM
      (fp16 products are exact in f32 -> E is full precision of the
      fp16-rounded inputs; no replicated-x DMA, VectorE stays idle).
    ScalarE: K = Exp(-g_c*E + bias_j) PSUM->SBUF fp16 contiguous,
      accum_out -> f32 partial row-sums.
    TensorE: per 8-i group, fp16 gram matmul accumulated into G_diag /
      G_off PSUM tiles; off-diag groups also get a [128,1] ones-matmul
      = column sums (the transposed halves of A, via symmetry).
  Host (f64): A = row-sums + col-sums, T = Gd + 2*Go, exact diagonal
  correction, HSIC combination. fp16 keeps T and A consistent enough
  that the unbiased-estimator cancellation survives (~2e-4 rel err).
"""

import sys
import numpy as np

if "/opt/trn_rl_repo" not in sys.path:
    sys.path.insert(0, "/opt/trn_rl_repo")

N = 2048
D = 16
P = 128
NB = N // P               # 16 block-rows
NCORES = 8
LA, LB = 9, 8             # blocks covered by row A / row B
FLA, FLB = LA * P, LB * P  # 1152, 1024
NQ = (LA - 1) * D + (LB - 1) * D  # 240 off-diag groups per core
NSLOT = 3                 # ACT accum slots: (A,0:1024), (B,0:1024), (A,1024:1152)

_NC_CACHE = {}


def _patch_tile_drain():
    """Walrus in this container accepts only 1 sync-wait per instruction.
    Tile routinely attaches several. Hoist extra waits onto single-wait NoOp
    carriers emitted just before the instruction on the same engine, and
    split the tail drain's per-engine waits the same way."""
    import concourse.mybir as mybir
    import concourse.tile as tile_mod
    from concourse.vector_clock import ScopedClock, VectorClock

    if getattr(tile_mod.TileContext, "_drain_patched", False):
        return

    orig_add = tile_mod.TileContext._add_instruction
    counter = [0]

    def _add_instruction(self, inst):
        si = inst.sync_info
        if si is not None and si.on_wait is not None and len(si.on_wait) > 1:
            waits = list(si.on_wait)
            for w in waits[:-1]:
                counter[0] += 1
                carrier = mybir.InstNoOp(name=f"waitc-{counter[0]}")
                carrier.engine = inst.engine
                carrier.sync_info = mybir.SyncInfo(on_wait=[w], on_update=[])
                orig_add(self, carrier)
            inst.sync_info = mybir.SyncInfo(
                on_wait=[waits[-1]], on_update=list(si.on_update or [])
            )
        orig_add(self, inst)

    def _drain_and_barrier(self, tick_clock, wait_clock):
        vec = list(tick_clock.global_clock)
        for i, v in enumerate(vec):
            if v <= 0:
                continue
            sub = [v if j == i else 0 for j in range(len(vec))]
            carrier = self.nc.sync.nop(nofuse=True)
            wait_clock.add_sem_waits(
                carrier.ins, ScopedClock({None: VectorClock(sub)})
            )
        self.nc.sync.drain()
        self.nc.all_engine_barrier()
        popped = self.nc._tile_sem_poison_stack.pop()
        assert popped is self._sem_poison
        # single-shot NEFF: skip the semaphore clear + second barrier
        # (they only matter if the program is re-executed on live sems)

    tile_mod.TileContext._add_instruction = _add_instruction
    tile_mod.TileContext._drain_and_barrier = _drain_and_barrier
    tile_mod.TileContext._drain_patched = True


def _build_nc():
    import concourse.bass as bass
    import concourse.mybir as mybir
    from concourse.tile import TileContext

    _patch_tile_drain()

    f32 = mybir.dt.float32
    f16 = mybir.dt.float16

    nc = bass.Bass("TRN2")
    xi_d = nc.dram_tensor("xi", [P, D * N], f16, kind="ExternalInput")
    scj_d = nc.dram_tensor("scj", [P, 2 * D], f16, kind="ExternalInput")
    bias_d = nc.dram_tensor("bias_t", [P, 2 * D], f32, kind="ExternalInput")
    gam_d = nc.dram_tensor("gam_t", [P, D], f32, kind="ExternalInput")
    ones_d = nc.dram_tensor("ones_t", [P, 1], f16, kind="ExternalInput")
    gout_d = nc.dram_tensor("gout", [P, 2 * P], f32, kind="ExternalOutput")
    cs_d = nc.dram_tensor("cs", [P, NQ], f32, kind="ExternalOutput")
    rsum_d = nc.dram_tensor("rsum", [P, NSLOT * D], f32, kind="ExternalOutput")

    FL = {0: FLA, 1: FLB}
    XO = {0: 0, 1: 1024}   # xi slab offset of each row's coverage window
    # (row, col_start, col_end, accum_slot) in emission order; last chunk is
    # the small one so its gram tail is short.
    chunks = [(0, 0, 1024, 0), (1, 0, 1024, 1), (0, 1024, FLA, 2)]

    with TileContext(nc) as tc:
        with (
            tc.tile_pool(name="const", bufs=1) as cpool,
            tc.tile_pool(name="e", bufs=2) as epool,
            tc.tile_pool(name="acc", bufs=1, space="PSUM") as apool,
        ):
            xi_sb = cpool.tile([P, D * N], f16)
            scj_sb = cpool.tile([P, 2 * D], f16)
            bias_sb = cpool.tile([P, 2 * D], f32)
            gam_sb = cpool.tile([P, D], f32)
            ones_sb = cpool.tile([P, 1], f16)
            ka_sb = cpool.tile([P, D * FLA], f16)
            kb_sb = cpool.tile([P, D * FLB], f16)
            rsum_sb = cpool.tile([P, NSLOT * D], f32)
            gout_sb = cpool.tile([P, 2 * P], f32)
            cs_sb = cpool.tile([P, NQ], f32)
            scr_sb = cpool.tile([P, 1], f32)

            nc.sync.dma_start(ones_sb[:], ones_d[:])
            nc.sync.dma_start(scj_sb[:], scj_d[:])
            # xi slabs split in halves, ordered by consumption: chunk A0
            # reads [0:1024) of each slab; the [1024:2048) halves (chunks
            # B0/A1/B1) stream later from the idle GpSimd DMA queue
            nc.sync.dma_start(xi_sb[:, 0:1024], xi_d[:, 0:1024])
            nc.sync.dma_start(bias_sb[:], bias_d[:])
            nc.sync.dma_start(gam_sb[:], gam_d[:])
            for c in range(1, D):
                nc.sync.dma_start(
                    xi_sb[:, c * N : c * N + 1024], xi_d[:, c * N : c * N + 1024]
                )
            for c in range(D):
                nc.sync.dma_start(
                    xi_sb[:, c * N + 1024 : (c + 1) * N],
                    xi_d[:, c * N + 1024 : (c + 1) * N],
                )

            # one accumulation region per 2KB PSUM bank: a start=True matmul
            # clears has_written for the whole bank, so interleaved
            # accumulation groups must not share banks (pad tiles to 512 f32)
            gdps = apool.tile([P, 512], f32)     # [:, :128] = G_diag
            gops = apool.tile([P, 512], f32)     # [:, :128] = G_off
            csps = apool.tile([P, 512], f32)     # [:, :NQ]  = col sums

            # early exp-table load (overlaps input DMA)
            nc.scalar.activation(
                out=scr_sb[:],
                in_=ones_sb[:],
                func=mybir.ActivationFunctionType.Exp,
            )

            ksb = {0: ka_sb, 1: kb_sb}
            n_diag = 2 * D                        # 32 diag gram matmuls
            n_off = NQ                            # 240 off gram matmuls
            di = [0]
            oi = [0]
            ei = [0]
            ebig = [None]

            for (row, s, e, slot) in chunks:
                fl = FL[row]
                w = e - s
                for c in range(D):
                    # VectorE builds E = (xi - 2*xj)*xi in fp16 (PE stays
                    # free for the gram/colsum matmuls). E tiles hold 8
                    # c-slots each: fewer tiles -> far fewer semaphores to
                    # allocate, wait on, and clear in the end-of-kernel drain
                    if ei[0] % 8 == 0:
                        ebig[0] = epool.tile([P, 8192], f16, name="ebig")
                    et = ebig[0][:, (ei[0] % 8) * 1024 : (ei[0] % 8) * 1024 + 1024]
                    ei[0] += 1
                    xi_c = xi_sb[:, c * N + XO[row] + s : c * N + XO[row] + e]
                    nc.vector.scalar_tensor_tensor(
                        out=et[:, 0:w],
                        in0=xi_c,
                        scalar=scj_sb[:, row * D + c : row * D + c + 1],
                        in1=xi_c,
                        op0=mybir.AluOpType.subtract,
                        op1=mybir.AluOpType.mult,
                    )
                    # K layout: col = g*128 + c*8 + ii (group-interleaved) so
                    # gram operands are contiguous 1-D 128-col slices; the ACT
                    # write scatters 8-elem (16B) runs.
                    k3 = ksb[row][:].rearrange("p (g x) -> p g x", x=D * 8)
                    nc.scalar.activation(
                        out=k3[:, s // 8 : e // 8, c * 8 : (c + 1) * 8],
                        in_=et[:, 0:w],
                        func=mybir.ActivationFunctionType.Exp,
                        bias=bias_sb[:, row * D + c : row * D + c + 1],
                        scale=gam_sb[:, c : c + 1],
                        accum_out=rsum_sb[:, slot * D + c : slot * D + c + 1],
                    )
                # gram + colsum matmuls; ready only once all 16 c are done,
                # so they execute during the NEXT chunk's ACT phase (PE has
                # nothing else queued)
                for g in range(s // 8, e // 8):
                    op = ksb[row][:, g * 128 : (g + 1) * 128]
                    if g < 16:
                        nc.tensor.matmul(
                            gdps[:, 0:P], lhsT=op, rhs=op,
                            start=(di[0] == 0), stop=(di[0] == n_diag - 1),
                            skip_group_check=True,
                        )
                        di[0] += 1
                        if di[0] == n_diag:
                            nc.vector.tensor_copy(
                                gout_sb[:, 0:P], gdps[:, 0:P]
                            )
                            nc.sync.dma_start(
                                gout_d[:, 0:P], gout_sb[:, 0:P]
                            )
                    else:
                        nc.tensor.matmul(
                            gops[:, 0:P], lhsT=op, rhs=op,
                            start=(oi[0] == 0), stop=(oi[0] == n_off - 1),
                            skip_group_check=True,
                        )
                        q = (g - 16) if row == 0 else (LA - 1) * D + (g - 16)
                        nc.tensor.matmul(
                            csps[:, q : q + 1], lhsT=op, rhs=ones_sb[:, 0:1],
                            start=True, stop=True, skip_group_check=True,
                        )
                        oi[0] += 1

            nc.vector.tensor_copy(gout_sb[:, P : 2 * P], gops[:, 0:P])
            nc.vector.tensor_copy(cs_sb[:], csps[:, 0:NQ])
            nc.sync.dma_start(gout_d[:, P : 2 * P], gout_sb[:, P : 2 * P])
            nc.gpsimd.dma_start(cs_d[:], cs_sb[:])
            nc.gpsimd.dma_start(rsum_d[:], rsum_sb[:])
    return nc


def _get_nc():
    if "nc" not in _NC_CACHE:
        _NC_CACHE["nc"] = _build_nc()
    return _NC_CACHE["nc"]


def _prep(X):
    """Host-side constants shared by in-map prep and combine."""
    Xd = X.astype(np.float64)
    meanD = 2.0 * (np.mean(Xd * Xd, axis=0) - np.mean(Xd, axis=0) ** 2)
    g32 = (1.0 / (2.0 * meanD)).astype(np.float32)       # [D]
    x16 = X.astype(np.float16).astype(np.float32)        # \tilde x
    xsq16 = (x16 * x16).astype(np.float16).astype(np.float32)
    return g32, x16, xsq16


def _make_in_maps(X):
    _COMBINE_X[0] = np.ascontiguousarray(np.asarray(X, dtype=np.float32))
    g32, x16, xsq16 = _prep(X)
    bias_full = -(g32[None, :] * xsq16).astype(np.float32)   # [N, D]

    in_maps = []
    for r in range(NCORES):
        rows = [(r, FLA), (r + 8, FLB)]
        # xi: per feature, the full wrapped circle starting at r*P,
        # replicated across partitions (row A reads [0:1152), row B
        # [1024:2048) of each slab)
        idx = (r * P + np.arange(N)) % N
        xi = np.ascontiguousarray(
            np.broadcast_to(
                x16[idx, :].T.reshape(1, D * N).astype(np.float16), (P, D * N)
            )
        )
        scj = np.zeros((P, 2 * D), dtype=np.float16)
        bias = np.zeros((P, 2 * D), dtype=np.float32)
        for row, (J, fl) in enumerate(rows):
            jidx = J * P + np.arange(P)
            for c in range(D):
                scj[:, row * D + c] = 2.0 * x16[jidx, c]
                bias[:, row * D + c] = bias_full[jidx, c]
        gam = np.ascontiguousarray(
            np.broadcast_to(-g32[None, :], (P, D))
        ).astype(np.float32)
        ones = np.ones((P, 1), dtype=np.float16)
        in_maps.append(
            {"xi": xi, "scj": scj, "bias_t": bias, "gam_t": gam, "ones_t": ones}
        )
    return in_maps


def _combine(results, X=None):
    if X is None:
        X = _COMBINE_X[0]
    g32, x16, xsq16 = _prep(X)
    g64 = g32.astype(np.float64)

    # exact diagonal model: E_ii = -2*x^2 + q(x^2) (f32-exact products),
    # arg = fma(E, -g, -g*q(x^2)), K_ii = exp(arg)
    E_ii = (-2.0 * (x16.astype(np.float64) ** 2) + xsq16).astype(np.float32)
    bias_full = -(g32[None, :] * xsq16).astype(np.float32)
    arg = (
        -g64[None, :] * E_ii.astype(np.float64) + bias_full.astype(np.float64)
    ).astype(np.float32)
    Kii = np.exp(arg.astype(np.float64))                  # [N, D]

    A = np.zeros((D, N), dtype=np.float64)
    Tp = np.zeros((D, D), dtype=np.float64)
    for r in range(NCORES):
        res = results[r]
        rsum = res["rsum"].astype(np.float64)             # [P, 3*D]
        cs = res["cs"].astype(np.float64)                 # [P, 240]
        gout = res["gout"].astype(np.float64)             # [P, 256]
        rows = [(r, FLA), (r + 8, FLB)]
        # row-sums: slots 0,2 -> row A; slot 1 -> row B
        A[:, r * P : (r + 1) * P] += (rsum[:, 0:D] + rsum[:, 2 * D : 3 * D]).T
        A[:, (r + 8) * P : (r + 9) * P] += rsum[:, D : 2 * D].T
        # col-sums: partition p = c*8 + ii; col q = off-group index
        csv = cs.reshape(D, 8, NQ)                        # [c, ii, q]
        for row, (J, fl) in enumerate(rows):
            nq = (LA - 1) * D if row == 0 else (LB - 1) * D
            qb = 0 if row == 0 else (LA - 1) * D
            q0 = np.arange(nq)
            ii = np.arange(8)
            cols = P + q0[:, None] * 8 + ii[None, :]      # [nq, 8] within-row col
            idx = (J * P + cols) % N
            A[:, idx.ravel()] += csv[:, :, qb : qb + nq].transpose(
                0, 2, 1
            ).reshape(D, -1)
        gd = gout[:, :P].reshape(D, 8, D, 8)
        go = gout[:, P:].reshape(D, 8, D, 8)
        Tp += np.einsum("aibi->ab", gd) + 2.0 * np.einsum("aibi->ab", go)

    A -= Kii.T
    T = Tp - Kii.T @ Kii
    S = A.sum(axis=1)
    Dm = A @ A.T
    c0 = 1.0 / (N * (N - 3))
    hsic = c0 * (
        T + np.outer(S, S) / ((N - 1.0) * (N - 2.0)) - (2.0 / (N - 2.0)) * Dm
    )
    iu = np.triu_indices(D, 1)
    return np.float32(np.sum(hsic[iu] ** 2))


_COMBINE_X = [None]


def run_spmd(in_maps, **kwargs):
    from concourse import bass_utils

    nc = _get_nc()
    return bass_utils.run_bass_kernel_spmd(
        nc, in_maps, core_ids=list(range(NCORES)), **kwargs
    )


def kernel(X):
    X = np.ascontiguousarray(np.asarray(X, dtype=np.float32))
    _COMBINE_X[0] = X
    in_maps = _make_in_maps(X)
    res = run_spmd(in_maps)
    return _combine(res.results, X)
